# revision 1
# baseline (speedup 1.0000x reference)
"""GCN binding-affinity model on 8 TRN2 cores — v2.

Structural changes vs v1 (kernel.py):
  * L1 aggregation needs NO device gather: per-edge x[src] / deg[src] are
    shipped as host-sharded edge data (input sharding), normalized and
    scatter-summed on device (F=1 one-hot matmuls).
  * h1 = relu(outer(a, w1)) is exactly rank 2 (b1 == 0 per problem spec):
    h1 = relu(a) w1+ + relu(-a) w1-.  So L2's message table is just the two
    scalars s+/- = dinv * relu(+-a) per node -> AllGather 400KB instead of
    12.8MB; the 64-wide expansion happens AFTER aggregation via the fixed
    vectors u = relu(w1)@W2, v = relu(-w1)@W2.
  * Self-loop contributions are applied locally (not as gathered edges).
  * Exact per-window tile packing (variable ntA/ntB per window) instead of
    global caps -> ~20% fewer gather descriptors / matmuls / one-hots.
  * Gather tables are [*, 64] f32 rows (256B, the SWDGE minimum); for the
    scalar L2 table only cols 0:2 hold data (lhsT never reads the rest).

Math identical to reference:
  per layer: agg = dinv (.) ((A+I) (dinv (.) t)), dinv = rsqrt(indeg+1).
  L1 t = x (scalar), L2 t = h1 (rank 2 -> 2 scalar channels), L3 t = h2@W3.
"""

import os
import sys
from contextlib import ExitStack

import numpy as np

for _p in ("/opt/trn_rl_repo",):
    if _p not in sys.path and os.path.isdir(_p):
        sys.path.insert(0, _p)

import concourse.bass as bass
import concourse.mybir as mybir
import concourse.tile as tile
from concourse import bacc
from concourse import bass_utils
from concourse.masks import make_identity
from concourse.tile_rust import add_dep_helper

F32 = mybir.dt.float32
I16 = mybir.dt.int16
AF = mybir.ActivationFunctionType
OP = mybir.AluOpType

N_NODES = 50000
N_EDGES = 600000
N_GRAPHS = 256
C = 8
NW = 49
NWS = NW * 128          # 6272 nodes per core
PAD_N = C * NWS         # 50176
HALF = 32768
GP = 256
GSZ = 4                 # windows per gather group


def wrap16(vals):
    return np.ascontiguousarray(vals.reshape(-1, 16).T.astype(np.int16))


def rep8(block):
    return np.tile(block, (8, 1))


class Plan:
    """Per-core edge packing metadata (topology-derived, shared across cores
    as parallel lists)."""

    def __init__(self, edge_index):
        src = np.asarray(edge_index[0], dtype=np.int64)
        dst = np.asarray(edge_index[1], dtype=np.int64)
        self.deg = np.bincount(dst, minlength=N_NODES).astype(np.float32)

        core_of = dst // NWS
        order0 = np.argsort(core_of, kind="stable")
        src, dst, core_sorted = src[order0], dst[order0], core_of[order0]

        self.cores = []
        for c in range(C):
            base = c * NWS
            lo = np.searchsorted(core_sorted, c, side="left")
            hi = np.searchsorted(core_sorted, c, side="right")
            c_src, c_dst = src[lo:hi], dst[lo:hi]
            w_of = (c_dst - base) >> 7
            is_a = c_src < HALF
            key = w_of * 2 + (~is_a)
            order = np.argsort(key, kind="stable")
            c_src, c_dst, w_of, is_a, key = (
                c_src[order], c_dst[order], w_of[order], is_a[order],
                key[order])
            kstart = np.searchsorted(key, np.arange(2 * NW), side="left")
            kend = np.searchsorted(key, np.arange(2 * NW), side="right")
            cnt = kend - kstart
            cntA, cntB = cnt[0::2], cnt[1::2]
            ntA = np.ceil(cntA / 128).astype(int)
            ntB = np.ceil(cntB / 128).astype(int)
            nt = ntA + ntB
            rank = np.arange(len(key)) - kstart[key]  # rank within (w, half)

            tbase = np.concatenate([[0], np.cumsum(nt)]).astype(int)
            T_total = int(tbase[-1])

            # groups
            groups = []
            offA = [0]
            offB = [0]
            for ws in range(0, NW, GSZ):
                gn = min(GSZ, NW - ws)
                wl = np.arange(ws, ws + gn)
                gA = int(ntA[wl].sum())
                gB = int(ntB[wl].sum())
                cA0 = np.concatenate([[0], np.cumsum(ntA[wl])]).astype(int)
                cB0 = np.concatenate([[0], np.cumsum(ntB[wl])]).astype(int)
                groups.append(dict(ws=ws, gn=gn, gA=gA, gB=gB,
                                   cA0=cA0, cB0=cB0,
                                   ntA=ntA[wl].copy(), ntB=ntB[wl].copy(),
                                   colA=offA[-1], colB=offB[-1]))
                offA.append(offA[-1] + gA * 128 // 16)
                offB.append(offB[-1] + gB * 128 // 16)
            TOT_A, TOT_B = offA[-1], offB[-1]

            # per-edge global tile index + lane
            gt = np.where(
                is_a,
                tbase[w_of] + rank // 128,
                tbase[w_of] + ntA[w_of] + rank // 128)
            lane = rank % 128

            # per-edge position within the group idx sequences
            gi_of = w_of // GSZ
            grpA_off = np.zeros(NW, int)
            grpB_off = np.zeros(NW, int)
            for g in groups:
                ws, gn = g["ws"], g["gn"]
                grpA_off[ws:ws + gn] = g["cA0"][:gn] * 128
                grpB_off[ws:ws + gn] = g["cB0"][:gn] * 128
            colA_base = np.array([g["colA"] * 16 for g in groups])
            colB_base = np.array([g["colB"] * 16 for g in groups])
            posA = colA_base[gi_of] + grpA_off[w_of] + rank
            posB = colB_base[gi_of] + grpB_off[w_of] + rank

            seqA = np.zeros(TOT_A * 16, np.int64)
            seqB = np.zeros(TOT_B * 16, np.int64)
            seqA[posA[is_a]] = c_src[is_a]
            seqB[posB[~is_a]] = c_src[~is_a] - HALF

            self.cores.append(dict(
                base=base, src=c_src, dst=c_dst, w_of=w_of, is_a=is_a,
                rank=rank, ntA=ntA, ntB=ntB, nt=nt, tbase=tbase,
                T_total=T_total, groups=groups, TOT_A=TOT_A, TOT_B=TOT_B,
                gt=gt, lane=lane, seqA=seqA, seqB=seqB))

        self.maxT = max(cc["T_total"] for cc in self.cores)
        self.maxNT = max(int(cc["nt"].max()) for cc in self.cores)
        self.maxTOT_A = max(cc["TOT_A"] for cc in self.cores)
        self.maxTOT_B = max(cc["TOT_B"] for cc in self.cores)
        self.max_gC = max(g["gA"] + g["gB"]
                          for cc in self.cores for g in cc["groups"])
        # the device program is built once from core 0's plan shape; all
        # cores must share the same unrolled structure -> pad counts to the
        # max across cores.  Instead of that complexity: build per-core
        # programs?  SPMD needs ONE program.  So we equalize the plan:
        self._equalize()

    def _equalize(self):
        """Pad every core's per-window tile counts up to the max across
        cores so a single SPMD program fits all cores."""
        ntA_max = np.max([cc["ntA"] for cc in self.cores], axis=0)
        ntB_max = np.max([cc["ntB"] for cc in self.cores], axis=0)
        nt = ntA_max + ntB_max
        tbase = np.concatenate([[0], np.cumsum(nt)]).astype(int)
        T_total = int(tbase[-1])
        groups = []
        offA = [0]
        offB = [0]
        for ws in range(0, NW, GSZ):
            gn = min(GSZ, NW - ws)
            wl = np.arange(ws, ws + gn)
            gA = int(ntA_max[wl].sum())
            gB = int(ntB_max[wl].sum())
            cA0 = np.concatenate([[0], np.cumsum(ntA_max[wl])]).astype(int)
            cB0 = np.concatenate([[0], np.cumsum(ntB_max[wl])]).astype(int)
            groups.append(dict(ws=ws, gn=gn, gA=gA, gB=gB, cA0=cA0, cB0=cB0,
                               ntA=ntA_max[wl].copy(), ntB=ntB_max[wl].copy(),
                               colA=offA[-1], colB=offB[-1]))
            offA.append(offA[-1] + gA * 128 // 16)
            offB.append(offB[-1] + gB * 128 // 16)
        TOT_A, TOT_B = offA[-1], offB[-1]

        grpA_off = np.zeros(NW, int)
        grpB_off = np.zeros(NW, int)
        colA_base = np.zeros(NW, int)
        colB_base = np.zeros(NW, int)
        for g in groups:
            ws, gn = g["ws"], g["gn"]
            grpA_off[ws:ws + gn] = g["cA0"][:gn] * 128
            grpB_off[ws:ws + gn] = g["cB0"][:gn] * 128
            colA_base[ws:ws + gn] = g["colA"] * 16
            colB_base[ws:ws + gn] = g["colB"] * 16

        for cc in self.cores:
            w_of, is_a, rank = cc["w_of"], cc["is_a"], cc["rank"]
            gt = np.where(is_a,
                          tbase[w_of] + rank // 128,
                          tbase[w_of] + ntA_max[w_of] + rank // 128)
            posA = colA_base[w_of] + grpA_off[w_of] + rank
            posB = colB_base[w_of] + grpB_off[w_of] + rank
            seqA = np.zeros(TOT_A * 16, np.int64)
            seqB = np.zeros(TOT_B * 16, np.int64)
            seqA[posA[is_a]] = cc["src"][is_a]
            seqB[posB[~is_a]] = cc["src"][~is_a] - HALF
            cc.update(gt=gt, seqA=seqA, seqB=seqB)

        self.ntA = ntA_max
        self.ntB = ntB_max
        self.nt = nt
        self.tbase = tbase
        self.T_total = T_total
        self.groups = groups
        self.TOT_A = TOT_A
        self.TOT_B = TOT_B
        self.max_gC = max(g["gA"] + g["gB"] for g in groups)

    def key(self):
        return (self.T_total, self.TOT_A, self.TOT_B,
                tuple(self.nt.tolist()))


def prep_inputs(plan, x, W1, b1, W2, b2, W3, b3, lin1_w, lin1_b, lin2_w,
                lin2_b, edge_index, batch):
    assert not np.any(np.asarray(b1)), "v2 exploits b1 == 0 (rank-2 h1)"
    assert not np.any(np.asarray(b2)), "t3local exploits b2 == 0"
    x = np.asarray(x, np.float32).reshape(-1)
    batch = np.asarray(batch, dtype=np.int64)
    deg = plan.deg

    x_ext = np.zeros(PAD_N, np.float32); x_ext[:N_NODES] = x
    deg_ext = np.zeros(PAD_N, np.float32); deg_ext[:N_NODES] = deg
    batch_ext = np.full(PAD_N, -1.0, np.float32)
    batch_ext[:N_NODES] = batch.astype(np.float32)

    iota = np.broadcast_to(np.arange(GP, dtype=np.float32),
                           (128, GP)).copy()
    cnts = np.bincount(batch, minlength=GP).astype(np.float32)
    cnts2 = np.ascontiguousarray(cnts.reshape(2, 128).T)

    in_maps = []
    for c, cc in enumerate(plan.cores):
        T = plan.T_total
        drel = np.full((128, T), -1.0, np.float32)
        ysx = np.zeros((128, T), np.float32)
        ysd = np.zeros((128, T), np.float32)
        gt, lane = cc["gt"], cc["lane"]
        drel[lane, gt] = (cc["dst"] - cc["base"] - (cc["w_of"] << 7)
                          ).astype(np.float32)
        ysx[lane, gt] = x[cc["src"]]
        ysd[lane, gt] = deg[cc["src"]]

        ixa = rep8(wrap16(cc["seqA"]))
        ixb = rep8(wrap16(cc["seqB"]))

        base = cc["base"]
        sl = slice(base, base + NWS)
        nd = lambda a: np.ascontiguousarray(a[sl].reshape(NW, 128).T)
        degT = np.broadcast_to(deg_ext[sl][None, :], (128, NWS)).copy()

        in_maps.append({
            "ixa": ixa, "ixb": ixb,
            "drel": drel, "ysx": ysx, "ysd": ysd,
            "nd_batch": nd(batch_ext),
            "nd_x": nd(x_ext), "nd_deg": nd(deg_ext),
            "degT": degT,
            "iota": iota, "cnts": cnts2,
            "w1": np.asarray(W1, np.float32).reshape(1, 64),
            "w2": np.asarray(W2, np.float32).reshape(64, 128),
            "w3": np.asarray(W3, np.float32).reshape(128, 64),
            "b2": np.asarray(b2, np.float32).reshape(128, 1),
            "b3": np.asarray(b3, np.float32).reshape(64, 1),
            "l1w": np.asarray(lin1_w, np.float32).reshape(64, 32),
            "l1b": np.asarray(lin1_b, np.float32).reshape(32, 1),
            "l2w": np.asarray(lin2_w, np.float32).reshape(32, 1),
            "l2b": np.full((128, 1),
                           np.float32(np.asarray(lin2_b).reshape(())),
                           np.float32),
        })
    return in_maps


# ----------------------------------------------------------------------------
# Device program
# ----------------------------------------------------------------------------

def build_program(plan, reps=1, no_coll=False, no_gather=False,
                  no_mm=False, no_oh=False, t3local=False):
    rg = [list(range(C))]
    T = plan.T_total
    TOT_A, TOT_B = plan.TOT_A, plan.TOT_B
    MAXNT = int(plan.nt.max())
    MAXGC = plan.max_gC

    nc = bacc.Bacc("TRN2", target_bir_lowering=False, debug=False,
                   num_devices=C, num_swdge_queues=4)

    din = {}
    def inp(name, shape, dt=F32):
        din[name] = nc.dram_tensor(name, list(shape), dt,
                                   kind="ExternalInput")
        return din[name]

    inp("ixa", (128, TOT_A), I16)
    inp("ixb", (128, TOT_B), I16)
    inp("drel", (128, T)); inp("ysx", (128, T)); inp("ysd", (128, T))
    inp("nd_batch", (128, NW))
    inp("nd_x", (128, NW)); inp("nd_deg", (128, NW))
    inp("degT", (128, NWS))
    inp("iota", (128, GP)); inp("cnts", (128, 2))
    inp("w1", (1, 64)); inp("w2", (64, 128)); inp("w3", (128, 64))
    inp("b2", (128, 1)); inp("b3", (64, 1))
    inp("l1w", (64, 32)); inp("l1b", (32, 1)); inp("l2w", (32, 1))
    inp("l2b", (128, 1))

    out_d = nc.dram_tensor("out", [GP, 1], F32, kind="ExternalOutput")

    t2c_sl = nc.dram_tensor("t2c_slice", [NWS, 2], F32, kind="Internal")
    t2c_full = nc.dram_tensor("t2c_full", [PAD_N, 2], F32, kind="Internal",
                              addr_space="Shared")
    t2 = nc.dram_tensor("t2", [PAD_N, 64], F32, kind="Internal")
    y3_sl = nc.dram_tensor("y3_slice", [NWS, 64], F32, kind="Internal")
    y3_full = nc.dram_tensor("y3_full", [PAD_N, 64], F32, kind="Internal",
                             addr_space="Shared")
    t3c_sl = nc.dram_tensor("t3c_slice", [2, NWS], F32, kind="Internal")
    t3c_full = nc.dram_tensor("t3c_full", [2 * C, NWS], F32, kind="Internal",
                              addr_space="Shared")
    t3 = nc.dram_tensor("t3", [PAD_N, 64], F32, kind="Internal")
    pool_in = nc.dram_tensor("pool_in", [GP, 64], F32, kind="Internal")
    pool_out = nc.dram_tensor("pool_out", [GP, 64], F32, kind="Internal",
                              addr_space="Shared")

    with tile.TileContext(nc) as tc, ExitStack() as ctx:
        P = ctx.enter_context
        setup = P(tc.tile_pool(name="setup", bufs=1))
        oh_pool = P(tc.tile_pool(name="oh", bufs=2))
        gb_pool = P(tc.tile_pool(name="gb", bufs=2))
        fn_pool = P(tc.tile_pool(name="fn", bufs=2))
        psS = P(tc.tile_pool(name="psS", bufs=2, space="PSUM"))
        psZ = P(tc.tile_pool(name="psZ", bufs=2, space="PSUM"))
        psT = P(tc.tile_pool(name="psT", bufs=2, space="PSUM"))
        psHold = P(tc.tile_pool(name="psHold", bufs=1, space="PSUM"))
        ev1 = P(tc.tile_pool(name="ev1", bufs=3))
        ev2 = P(tc.tile_pool(name="ev2", bufs=3))
        ev3 = P(tc.tile_pool(name="ev3", bufs=3))
        stg = P(tc.tile_pool(name="stg", bufs=1))
        expp = P(tc.tile_pool(name="expp", bufs=2))

        def load(name, shape, dt=F32):
            t = setup.tile(list(shape), dt, tag=name)
            nc.sync.dma_start(out=t[:], in_=din[name].ap()[:])
            return t

        ixa = load("ixa", (128, TOT_A), I16)
        ixb = load("ixb", (128, TOT_B), I16)
        drel = load("drel", (128, T))
        ysx = load("ysx", (128, T))
        ysd = load("ysd", (128, T))
        nd_batch = load("nd_batch", (128, NW))
        nd_x = load("nd_x", (128, NW))
        nd_deg = load("nd_deg", (128, NW))
        degT = load("degT", (128, NWS))
        iota = load("iota", (128, GP))
        cnts = load("cnts", (128, 2))
        w1 = load("w1", (1, 64)); w2 = load("w2", (64, 128))
        w3 = load("w3", (128, 64))
        b2 = load("b2", (128, 1)); b3 = load("b3", (64, 1))
        l1w = load("l1w", (64, 32)); l1b = load("l1b", (32, 1))
        l2w = load("l2w", (32, 1)); l2b = load("l2b", (128, 1))

        ident = setup.tile([128, 128], F32, tag="ident")
        make_identity(nc, ident[:])
        fence_ix = setup.tile([128, 8], I16, tag="fence_ix")
        nc.vector.memset(fence_ix[:], 0)

        # dinvT = rsqrt(degT + 1) computed in place (free dim layout)
        dinvT = degT
        nc.scalar.activation(out=dinvT[:], in_=degT[:], func=AF.Sqrt,
                             bias=1.0, scale=1.0)
        nc.vector.reciprocal(out=dinvT[:], in_=dinvT[:])

        # per-slot src normalization: ys = x[src] * rsqrt(deg[src]+1)
        nc.scalar.activation(out=ysd[:], in_=ysd[:], func=AF.Sqrt,
                             bias=1.0, scale=1.0)
        nc.vector.reciprocal(out=ysd[:], in_=ysd[:])
        ys = ysx
        nc.vector.tensor_tensor(out=ys[:], in0=ysd[:], in1=ysx[:],
                                op=OP.mult)

        # own-node x*dinv, node-partition layout (L1 self-loop lhsT)
        dinv_nm = setup.tile([128, NW], F32, tag="dinv_nm")
        nc.scalar.activation(out=dinv_nm[:], in_=nd_deg[:], func=AF.Sqrt,
                             bias=1.0, scale=1.0)
        nc.vector.reciprocal(out=dinv_nm[:], in_=dinv_nm[:])
        xd_nm = setup.tile([128, NW], F32, tag="xd_nm")
        nc.vector.tensor_tensor(out=xd_nm[:], in0=nd_x[:], in1=dinv_nm[:],
                                op=OP.mult)

        # UVT = [u; v] = [relu(w1); relu(-w1)] @ W2  -> [2, 128]
        w1p = setup.tile([1, 64], F32, tag="w1p")
        w1m = setup.tile([1, 64], F32, tag="w1m")
        nc.scalar.activation(out=w1p[:], in_=w1[:], func=AF.Relu)
        nc.scalar.activation(out=w1m[:], in_=w1[:], func=AF.Relu,
                             scale=-1.0)
        w1pmT = setup.tile([64, 2], F32, tag="w1pmT")
        for i, src_t in enumerate((w1p, w1m)):
            psx = psT.tile([128, 128], F32, space="PSUM", tag="psN")
            nc.tensor.transpose(out=psx[:64, :1], in_=src_t[:],
                                identity=ident[:1, :1])
            nc.scalar.activation(out=w1pmT[:, i:i + 1], in_=psx[:64, :1],
                                 func=AF.Copy)
        psUV = psZ.tile([128, 256], F32, space="PSUM", tag="psz")
        nc.tensor.matmul(out=psUV[:2, :128], lhsT=w1pmT[:], rhs=w2[:],
                         start=True, stop=True)
        UVT = setup.tile([2, 128], F32, tag="UVT")
        nc.scalar.activation(out=UVT[:], in_=psUV[:2, :128], func=AF.Copy)

        staging = stg.tile([128, NW * 64], F32, tag="staging")
        staging2 = stg.tile([128, NW * 2], F32, tag="staging2")
        if t3local:
            staging2c = stg.tile([2, NWS], F32, tag="staging2c")
        else:
            staging2c = None

        def wsl(w):
            return slice(w * 128, (w + 1) * 128)

        def onehot_win(w):
            ntw = int(plan.nt[w])
            t0 = int(plan.tbase[w])
            oh = oh_pool.tile([128, MAXNT * 128], F32, tag="oh")
            if no_oh or ntw == 0:
                nc.vector.memset(oh[:], 0.0)
                return oh
            dr3 = drel[:, t0:t0 + ntw][:, :, None].to_broadcast(
                [128, ntw, 128])
            io3 = iota[:, None, :128].to_broadcast([128, ntw, 128])
            nc.vector.tensor_tensor(
                out=oh[:, :ntw * 128].rearrange("p (t j) -> p t j", j=128),
                in0=dr3, in1=io3, op=OP.is_equal)
            return oh

        def gather_group(g, table):
            gb = gb_pool.tile([128, MAXGC * 64], F32, tag="gb")
            if no_gather:
                nc.vector.memset(gb[:], 0.0)
                return gb, None
            # split each half across two SWDGE queues (4 total)
            qcalls = {0: [], 1: [], 2: [], 3: []}

            def issue(base_chunk, nchunks, colbase, table_ap, queues):
                if nchunks == 0:
                    return
                h1 = (nchunks // 2) if nchunks > 1 else nchunks
                parts = [(0, h1), (h1, nchunks - h1)] if nchunks > 1 \
                    else [(0, nchunks)]
                for (c0, nc_), qn in zip(parts, queues):
                    if nc_ == 0:
                        continue
                    n = nc_ * 128
                    qcalls[qn].append(nc.gpsimd.dma_gather(
                        out_ap=gb[:, (base_chunk + c0) * 64:
                                  (base_chunk + c0 + nc_) * 64].rearrange(
                            "p (t f) -> p t f", f=64),
                        in_ap=table_ap,
                        idxs_ap=ixa[:, 0:1] if False else
                        _ix[:, colbase + c0 * 8:colbase + (c0 + nc_) * 8],
                        num_idxs=n, num_idxs_reg=n, elem_size=64,
                        single_packet=False, queue_num=qn))

            _ix = ixa
            issue(0, g["gA"], g["colA"], table.ap()[:HALF, :], (0, 2))
            _ix = ixb
            issue(g["gA"], g["gB"], g["colB"], table.ap()[HALF:, :], (1, 3))

            fences = []
            for qn in range(4):
                if not qcalls[qn]:
                    continue
                fence_t = fn_pool.tile([128, 64], F32, tag=f"fence{qn}")
                f = nc.gpsimd.dma_gather(
                    out_ap=fence_t[:].rearrange("p (t f) -> p t f", f=64),
                    in_ap=table.ap()[:HALF, :],
                    idxs_ap=fence_ix[:],
                    num_idxs=128, num_idxs_reg=128, elem_size=64,
                    single_packet=True, queue_num=qn)
                for call in qcalls[qn]:
                    add_dep_helper(f.ins, call.ins, True, f"f{qn}>call")
                fences.append(f)
            fence = fences[0]
            for f in fences[1:]:
                add_dep_helper(fence.ins, f.ins, True, "fence>f")
            return gb, fence

        def gb_col(g, wl, t):
            """gb chunk index for window-in-group wl, window tile t."""
            ntAw = int(g["ntA"][wl])
            if t < ntAw:
                return int(g["cA0"][wl]) + t
            return g["gA"] + int(g["cB0"][wl]) + (t - ntAw)

        def scatter_win(oh, lhs_cols, fence, F, self_lhs=None):
            """lhs_cols: (buf, col) per gather tile; self_lhs: (buf, col)
            whose rhs is the identity (adds own-node values)."""
            ps = psS.tile([64, 128], F32, space="PSUM", tag="psS")
            if no_mm:
                nc.vector.memset(ps[:], 0.0)
                return ps
            n = len(lhs_cols) + (1 if self_lhs is not None else 0)
            for t, (buf, col) in enumerate(lhs_cols):
                mm = nc.tensor.matmul(
                    out=ps[:F, :], lhsT=buf[:, col:col + F],
                    rhs=oh[:, t * 128:(t + 1) * 128],
                    start=(t == 0), stop=(t == n - 1))
                if fence is not None:
                    add_dep_helper(mm.ins, fence.ins, True, "mm>fence")
            if self_lhs is not None:
                buf, col = self_lhs
                nc.tensor.matmul(
                    out=ps[:F, :], lhsT=buf[:, col:col + F],
                    rhs=ident[:128, :128], start=(len(lhs_cols) == 0),
                    stop=True)
            return ps

        for _rep in range(reps):
            # ---- Layer 1 (no gather) ---------------------------------------
            for w in range(NW):
                oh = onehot_win(w)
                cols = [(ys, int(plan.tbase[w]) + t)
                        for t in range(int(plan.nt[w]))]
                ps1 = scatter_win(oh, cols, None, 1, self_lhs=(xd_nm, w))
                a = ev1.tile([1, 128], F32, tag="a")
                nc.vector.tensor_tensor(out=a[:], in0=ps1[:1, :],
                                        in1=dinvT[:1, wsl(w)], op=OP.mult)
                sp = ev2.tile([1, 128], F32, tag="sp")
                nc.scalar.activation(out=sp[:], in_=a[:], func=AF.Relu)
                sm = ev2.tile([1, 128], F32, tag="sm")
                nc.scalar.activation(out=sm[:], in_=a[:], func=AF.Relu,
                                     scale=-1.0)
                spd = ev3.tile([1, 128], F32, tag="spd")
                nc.vector.tensor_tensor(out=spd[:], in0=sp[:],
                                        in1=dinvT[:1, wsl(w)], op=OP.mult)
                smd = ev3.tile([1, 128], F32, tag="smd")
                nc.vector.tensor_tensor(out=smd[:], in0=sm[:],
                                        in1=dinvT[:1, wsl(w)], op=OP.mult)
                psN = psT.tile([128, 128], F32, space="PSUM", tag="psN")
                nc.tensor.transpose(out=psN[:, 0:1], in_=spd[:],
                                    identity=ident[:1, :1])
                nc.tensor.transpose(out=psN[:, 1:2], in_=smd[:],
                                    identity=ident[:1, :1])
                nc.scalar.activation(out=staging2[:, w * 2:(w + 1) * 2],
                                     in_=psN[:, :2], func=AF.Copy)

            nc.sync.dma_start(
                out=t2c_sl.ap()[:].rearrange("(w p) f -> p w f", p=128),
                in_=staging2[:].rearrange("p (w f) -> p w f", f=2))
            if no_coll:
                nc.gpsimd.dma_start(out=t2c_full.ap()[:NWS, :],
                                    in_=t2c_sl.ap()[:])
            else:
                nc.gpsimd.collective_compute(
                    "AllGather", OP.bypass, replica_groups=rg,
                    ins=[t2c_sl.ap()[:]], outs=[t2c_full.ap()[:]])
            # expand compact [N,2] into full 256B rows (s+,s- repeated) via
            # SBUF bounce -- avoids a 50k-descriptor strided DRAM write
            t2sb = stg.tile([128, 2 * PAD_N // 128], F32, tag="t2sb")
            nc.sync.dma_start(
                out=t2sb[:].rearrange("p (w f) -> p w f", f=2),
                in_=t2c_full.ap()[:].rearrange("(w p) f -> p w f", p=128))
            for ch in range(8):
                exp = expp.tile([128, NW * 64], F32, tag="exp")
                seg = t2sb[:, ch * 2 * NW:(ch + 1) * 2 * NW].rearrange(
                    "p (w f) -> p w f", f=2)
                nc.vector.tensor_copy(
                    out=exp[:].rearrange("p (w d f) -> p w d f", d=32, f=2),
                    in_=seg[:, :, None, :].to_broadcast([128, NW, 32, 2]))
                nc.sync.dma_start(
                    out=t2.ap()[ch * NWS:(ch + 1) * NWS, :].rearrange(
                        "(w p) f -> p w f", p=128),
                    in_=exp[:].rearrange("p (w f) -> p w f", f=64))

            # ---- Layer 2 ---------------------------------------------------
            for g in plan.groups:
                gb, fence = gather_group(g, t2)
                for wl in range(g["gn"]):
                    w = g["ws"] + wl
                    oh = onehot_win(w)
                    cols = [(gb, gb_col(g, wl, t) * 64)
                            for t in range(int(plan.nt[w]))]
                    ps2 = scatter_win(oh, cols, fence, 2,
                                      self_lhs=(staging2, w * 2))
                    pq = ev1.tile([2, 128], F32, tag="pq")
                    nc.vector.tensor_tensor(out=pq[:], in0=ps2[:2, :],
                                            in1=dinvT[:2, wsl(w)],
                                            op=OP.mult)
                    if t3local:
                        # p~,q~ = dinv * (p,q); table row built locally later
                        nc.vector.tensor_tensor(out=staging2c[:, wsl(w)],
                                                in0=pq[:],
                                                in1=dinvT[:2, wsl(w)],
                                                op=OP.mult)
                    psh = psZ.tile([128, 256], F32, space="PSUM", tag="psz")
                    nc.tensor.matmul(out=psh[:, :128], lhsT=UVT[:], rhs=pq[:],
                                     start=True, stop=True)
                    h2 = ev2.tile([128, 128], F32, tag="h2")
                    nc.scalar.activation(out=h2[:], in_=psh[:, :128],
                                         func=AF.Relu, bias=b2[:])
                    pst3 = psZ.tile([128, 256], F32, space="PSUM", tag="psz")
                    nc.tensor.matmul(out=pst3[:64, :128], lhsT=w3[:],
                                     rhs=h2[:], start=True, stop=True)
                    g3 = ev3.tile([64, 128], F32, tag="g3")
                    nc.vector.tensor_tensor(out=g3[:],
                                            in0=pst3[:64, :128],
                                            in1=dinvT[:64, wsl(w)],
                                            op=OP.mult)
                    psN = psT.tile([128, 128], F32, space="PSUM", tag="psN")
                    nc.tensor.transpose(out=psN[:, :64], in_=g3[:],
                                        identity=ident[:64, :64])
                    nc.scalar.activation(out=staging[:, w * 64:(w + 1) * 64],
                                         in_=psN[:, :64], func=AF.Copy)

            if t3local:
                nc.sync.dma_start(out=t3c_sl.ap()[:], in_=staging2c[:])
                if no_coll:
                    nc.gpsimd.dma_start(out=t3c_full.ap()[:2, :],
                                        in_=t3c_sl.ap()[:])
                else:
                    nc.gpsimd.collective_compute(
                        "AllGather", OP.bypass, replica_groups=rg,
                        ins=[t3c_sl.ap()[:]], outs=[t3c_full.ap()[:]])
                # local T3 build: row n = relu(p~ u + q~ v) @ W3  (b2 == 0)
                for c8 in range(C):
                    pqsb = stg.tile([2, NWS], F32, tag="pqsb")
                    nc.sync.dma_start(out=pqsb[:],
                                      in_=t3c_full.ap()[2 * c8:2 * c8 + 2, :])
                    tstag = expp.tile([128, NW * 64], F32, tag="exp")
                    for wc in range(NW):
                        psh = psZ.tile([128, 256], F32, space="PSUM",
                                       tag="psz")
                        nc.tensor.matmul(out=psh[:, :128], lhsT=UVT[:],
                                         rhs=pqsb[:, wc * 128:(wc + 1) * 128],
                                         start=True, stop=True)
                        h2b = ev2.tile([128, 128], F32, tag="h2")
                        nc.scalar.activation(out=h2b[:], in_=psh[:, :128],
                                             func=AF.Relu)
                        pst = psZ.tile([128, 256], F32, space="PSUM",
                                       tag="psz")
                        nc.tensor.matmul(out=pst[:64, :128], lhsT=w3[:],
                                         rhs=h2b[:], start=True, stop=True)
                        t3v = ev3.tile([64, 128], F32, tag="g3")
                        nc.scalar.activation(out=t3v[:], in_=pst[:64, :128],
                                             func=AF.Copy)
                        psN = psT.tile([128, 128], F32, space="PSUM",
                                       tag="psN")
                        nc.tensor.transpose(out=psN[:, :64], in_=t3v[:],
                                            identity=ident[:64, :64])
                        nc.scalar.activation(
                            out=tstag[:, wc * 64:(wc + 1) * 64],
                            in_=psN[:, :64], func=AF.Copy)
                    nc.sync.dma_start(
                        out=t3.ap()[c8 * NWS:(c8 + 1) * NWS, :].rearrange(
                            "(w p) f -> p w f", p=128),
                        in_=tstag[:].rearrange("p (w f) -> p w f", f=64))
            else:
                nc.sync.dma_start(
                    out=y3_sl.ap()[:].rearrange("(w p) f -> p w f", p=128),
                    in_=staging[:].rearrange("p (w f) -> p w f", f=64))
                if no_coll:
                    nc.gpsimd.dma_start(out=y3_full.ap()[:NWS, :],
                                        in_=y3_sl.ap()[:])
                else:
                    nc.gpsimd.collective_compute(
                        "AllGather", OP.bypass, replica_groups=rg,
                        ins=[y3_sl.ap()[:]], outs=[y3_full.ap()[:]])

            # ---- Layer 3 + pooling ----------------------------------------
            pooled_a = psHold.tile([128, 64], F32, space="PSUM", tag="pool_a")
            pooled_b = psHold.tile([128, 64], F32, space="PSUM", tag="pool_b")
            for g in plan.groups:
                gb, fence = gather_group(g, t3 if t3local else y3_full)
                for wl in range(g["gn"]):
                    w = g["ws"] + wl
                    oh = onehot_win(w)
                    cols = [(gb, gb_col(g, wl, t) * 64)
                            for t in range(int(plan.nt[w]))]
                    ps3 = scatter_win(oh, cols, fence, 64,
                                      self_lhs=(staging, w * 64))
                    agg = ev1.tile([64, 128], F32, tag="agg")
                    nc.vector.tensor_tensor(out=agg[:], in0=ps3[:64, :],
                                            in1=dinvT[:64, wsl(w)],
                                            op=OP.mult)
                    h3 = ev2.tile([64, 128], F32, tag="h3")
                    nc.scalar.activation(out=h3[:], in_=agg[:], func=AF.Relu,
                                         bias=b3[:])
                    psN = psT.tile([128, 128], F32, space="PSUM", tag="psN")
                    nc.tensor.transpose(out=psN[:, :64], in_=h3[:],
                                        identity=ident[:64, :64])
                    h3nm = ev3.tile([128, 64], F32, tag="h3nm")
                    nc.scalar.activation(out=h3nm[:], in_=psN[:, :64],
                                         func=AF.Copy)
                    ohp = oh_pool.tile([128, GP], F32, tag="ohp")
                    bc = nd_batch[:, w:w + 1].to_broadcast([128, GP])
                    nc.vector.tensor_tensor(out=ohp[:], in0=bc, in1=iota[:],
                                            op=OP.is_equal)
                    for half, ps_pool in ((0, pooled_a), (1, pooled_b)):
                        lhs = ohp[:, half * 128:(half + 1) * 128]
                        nc.tensor.matmul(out=ps_pool[:], lhsT=lhs,
                                         rhs=h3nm[:],
                                         start=(w == 0), stop=(w == NW - 1))

            # ---- finale ----------------------------------------------------
            pa = setup.tile([128, 64], F32, tag="pa")
            pb = setup.tile([128, 64], F32, tag="pb")
            nc.scalar.activation(out=pa[:], in_=pooled_a[:], func=AF.Copy)
            nc.scalar.activation(out=pb[:], in_=pooled_b[:], func=AF.Copy)
            nc.sync.dma_start(out=pool_in.ap()[0:128, :], in_=pa[:])
            nc.sync.dma_start(out=pool_in.ap()[128:256, :], in_=pb[:])
            if no_coll:
                nc.gpsimd.dma_start(out=pool_out.ap()[:],
                                    in_=pool_in.ap()[:])
            else:
                nc.gpsimd.collective_compute(
                    "AllReduce", OP.add, replica_groups=rg,
                    ins=[pool_in.ap()[:]], outs=[pool_out.ap()[:]])

        meanT = setup.tile([64, 256], F32, tag="meanT")
        for half in (0, 1):
            pl = setup.tile([128, 64], F32, tag=f"pl{half}")
            nc.sync.dma_start(
                out=pl[:], in_=pool_out.ap()[half * 128:(half + 1) * 128, :])
            cntm = setup.tile([128, 1], F32, tag=f"cntm{half}")
            nc.vector.tensor_scalar_max(out=cntm[:],
                                        in0=cnts[:, half:half + 1],
                                        scalar1=1.0)
            rc = setup.tile([128, 1], F32, tag=f"rc{half}")
            nc.vector.reciprocal(out=rc[:], in_=cntm[:])
            mean = setup.tile([128, 64], F32, tag=f"mean{half}")
            nc.vector.tensor_scalar_mul(out=mean[:], in0=pl[:],
                                        scalar1=rc[:])
            psMT = psT.tile([128, 128], F32, space="PSUM", tag="psN")
            nc.tensor.transpose(out=psMT[:64, :], in_=mean[:],
                                identity=ident[:])
            nc.scalar.activation(out=meanT[:, half * 128:(half + 1) * 128],
                                 in_=psMT[:64, :], func=AF.Copy)

        psZ1 = psZ.tile([128, 256], F32, space="PSUM", tag="psz")
        nc.tensor.matmul(out=psZ1[:32, :256], lhsT=l1w[:], rhs=meanT[:],
                         start=True, stop=True)
        z1 = setup.tile([32, 256], F32, tag="z1")
        nc.scalar.activation(out=z1[:], in_=psZ1[:32, :256], func=AF.Relu,
                             bias=l1b[:])
        for half in (0, 1):
            psO = psT.tile([128, 128], F32, space="PSUM", tag="psN")
            nc.tensor.matmul(out=psO[:, :1],
                             lhsT=z1[:, half * 128:(half + 1) * 128],
                             rhs=l2w[:], start=True, stop=True)
            ob = setup.tile([128, 1], F32, tag=f"ob{half}")
            nc.scalar.activation(out=ob[:], in_=psO[:, :1],
                                 func=AF.Identity, bias=l2b[:])
            nc.sync.dma_start(out=out_d.ap()[half * 128:(half + 1) * 128, :],
                              in_=ob[:])

    nc.compile()
    return nc


# ----------------------------------------------------------------------------
# Runner
# ----------------------------------------------------------------------------

_CACHE = {}


def get_program(plan, reps=1, **kw):
    key = plan.key() + (reps,) + tuple(sorted(kw.items()))
    if key not in _CACHE:
        _CACHE[key] = build_program(plan, reps, **kw)
    return _CACHE[key]


def run(plan, inputs, trace=False):
    in_maps = prep_inputs(plan, **inputs)
    nc = get_program(plan)
    res = bass_utils.run_bass_kernel_spmd(
        nc, in_maps, core_ids=list(range(C)), trace=trace)
    out = res.results[0]["out"][:N_GRAPHS, :].astype(np.float32)
    return out, res


def kernel(**inputs) -> np.ndarray:
    plan = Plan(inputs["edge_index"])
    out, _ = run(plan, inputs)
    return out

